# revision 1
# baseline (speedup 1.0000x reference)
"""CTBG circuit kernel for Trainium2, data-parallel over batch on 8 NeuronCores.

Network (per reference):
  gpe_out = x @ (gpe_w * gpe_mask.T) + gpe_b              [B, 1536]
  gpi_in  = concat([x, gpe_out], -1)                      [B, 3072]
  gpi_out = gpi_in @ (gpi_w * gpi_mask.T) + gpi_b         [B, 1536]
  h1 = relu(gpi_out @ w1 + b1); h2 = relu(h1 @ w2 + b2)
  out = relu(h2 @ w3 + b3)                                [B, 6]

Per-core dataflow (feature-major activations, bf16 compute, f32 accumulate):
  - stage x and masks to bf16 in DRAM (SWDGE cast-DMA), then DMA-xbar
    transpose slices into SBUF: xT [1536, 2048] tiles and maskT row-chunks.
  - masked weights materialized in SBUF once: mw = w_bf16 * maskT (in-place
    on the weight tiles, DVE).
  - chain of matmuls with contraction dim on partitions; activations stay
    feature-major so no transposes are needed between layers.
  - PSUM drained by ScalarE activation (bias add + relu/identity) directly
    to bf16 SBUF tiles feeding the next layer.
  - output produced as [6, 2048] f32 per core; host transposes + concats.
"""

import numpy as np

NCORES = 8
B = 16384
BS = B // NCORES          # 2048 rows per core
BT = 512                  # batch tile (matmul free dim)
NBT = BS // BT            # 4
D1 = 1536                 # gpe input dim
D2 = 1536                 # gpe output dim
D3 = 3072                 # gpi input dim
D4 = 1536                 # gpi output dim
H = 512                   # mlp hidden
A = 6                     # action dim

K1 = D1 // 128            # 12
U2 = D2 // 128            # 12
K3 = D3 // 128            # 24
V4 = D4 // 128            # 12
M5 = H // 128             # 4

_CACHE = {}


def _build():
    import concourse.bacc as bacc
    import concourse.tile as tile
    from concourse import mybir
    from concourse.masks import make_identity

    FP32 = mybir.dt.float32
    BF16 = mybir.dt.bfloat16
    Act = mybir.ActivationFunctionType

    nc = bacc.Bacc(None)

    x_d = nc.dram_tensor("x", [BS, D1], FP32, kind="ExternalInput")
    gpem_d = nc.dram_tensor("gpe_mask", [D2, D1], FP32, kind="ExternalInput")
    gpew_d = nc.dram_tensor("gpe_w", [D1, D2], FP32, kind="ExternalInput")
    gpeb_d = nc.dram_tensor("gpe_b", [D2], FP32, kind="ExternalInput")
    gpim_d = nc.dram_tensor("gpi_mask", [D4, D3], FP32, kind="ExternalInput")
    gpiw_d = nc.dram_tensor("gpi_w", [D3, D4], FP32, kind="ExternalInput")
    gpib_d = nc.dram_tensor("gpi_b", [D4], FP32, kind="ExternalInput")
    w1_d = nc.dram_tensor("w1", [D4, H], FP32, kind="ExternalInput")
    b1_d = nc.dram_tensor("b1", [H], FP32, kind="ExternalInput")
    w2_d = nc.dram_tensor("w2", [H, H], FP32, kind="ExternalInput")
    b2_d = nc.dram_tensor("b2", [H], FP32, kind="ExternalInput")
    w3_d = nc.dram_tensor("w3", [H, A], FP32, kind="ExternalInput")
    b3_d = nc.dram_tensor("b3", [A], FP32, kind="ExternalInput")
    o_d = nc.dram_tensor("out", [A, BS], FP32, kind="ExternalOutput")

    # bf16 DRAM staging for the tensors that need an xbar transpose
    x_bf = nc.dram_tensor("x_bf", [BS, D1], BF16)
    gpem_bf = nc.dram_tensor("gpem_bf", [D2, D1], BF16)
    gpim_bf = nc.dram_tensor("gpim_bf", [D4, D3], BF16)

    with tile.TileContext(nc) as tc:
        with (
            tc.tile_pool(name="wpool", bufs=1) as wp,        # persistent weights
            tc.tile_pool(name="mpool", bufs=3) as mp,        # transient maskT tiles
            tc.tile_pool(name="xpool", bufs=2) as xp,        # xT double-buffered
            tc.tile_pool(name="apool", bufs=1) as ap,        # activations
            tc.tile_pool(name="opool", bufs=2) as op,        # output staging
            tc.tile_pool(name="pspool", bufs=4, space="PSUM") as psp,
            tc.tile_pool(name="ps5pool", bufs=2, space="PSUM") as ps5p,
        ):
            # ---------------- SWDGE (gpsimd) queue: staging + cast loads ----
            # order matters: earliest-needed first
            nc.gpsimd.dma_start(out=x_bf[0:BT, :], in_=x_d[0:BT, :])
            nc.gpsimd.dma_start(out=gpem_bf[:, :], in_=gpem_d[:, :])
            wgpe = []
            for k in range(K1):
                t = wp.tile([128, D2], BF16, tag=f"wgpe{k}")
                nc.gpsimd.dma_start(out=t[:, :], in_=gpew_d[k * 128:(k + 1) * 128, :])
                wgpe.append(t)
            for t_i in range(1, NBT):
                nc.gpsimd.dma_start(out=x_bf[t_i * BT:(t_i + 1) * BT, :],
                                    in_=x_d[t_i * BT:(t_i + 1) * BT, :])
            nc.gpsimd.dma_start(out=gpim_bf[:, :], in_=gpim_d[:, :])
            wgpi = []
            for k in range(K3):
                t = wp.tile([128, D4], BF16, tag=f"wgpi{k}")
                nc.gpsimd.dma_start(out=t[:, :], in_=gpiw_d[k * 128:(k + 1) * 128, :])
                wgpi.append(t)
            w1s = []
            for k in range(V4):
                t = wp.tile([128, H], BF16, tag=f"w1_{k}")
                nc.gpsimd.dma_start(out=t[:, :], in_=w1_d[k * 128:(k + 1) * 128, :])
                w1s.append(t)
            w2s = []
            for k in range(M5):
                t = wp.tile([128, H], BF16, tag=f"w2_{k}")
                nc.gpsimd.dma_start(out=t[:, :], in_=w2_d[k * 128:(k + 1) * 128, :])
                w2s.append(t)
            w3s = []
            for k in range(M5):
                t = wp.tile([128, A], BF16, tag=f"w3_{k}")
                nc.gpsimd.dma_start(out=t[:, :], in_=w3_d[k * 128:(k + 1) * 128, :])
                w3s.append(t)

            # ---------------- biases: natural load + PE transpose ----------
            ident = wp.tile([128, 128], FP32, tag="ident")
            make_identity(nc, ident[:, :])

            def load_bias(b_dram, n, tag):
                # b [n*128] -> SBUF [128, n] via PE transpose
                nat = wp.tile([max(n, 1), 128], FP32, tag=f"{tag}_nat")
                nc.scalar.dma_start(out=nat[:, :],
                                    in_=b_dram.rearrange("(c p) -> c p", p=128))
                ps = ps5p.tile([128, max(n, 1)], FP32, tag="ps_bias")
                nc.tensor.transpose(ps[:, :], nat[:, :], ident[0:n, 0:n])
                sb = wp.tile([128, max(n, 1)], FP32, tag=tag)
                nc.vector.tensor_copy(sb[:, :], ps[:, :])
                return sb

            gpeb_sb = load_bias(gpeb_d, U2, "gpeb")
            gpib_sb = load_bias(gpib_d, V4, "gpib")
            b1_sb = load_bias(b1_d, M5, "b1sb")
            b2_sb = load_bias(b2_d, M5, "b2sb")
            b3_sb = wp.tile([A, 1], FP32, tag="b3sb")
            nc.scalar.dma_start(out=b3_sb[:, :],
                                in_=b3_d.rearrange("(a one) -> a one", one=1))

            # ---------------- SP queue: xbar transposes --------------------
            def load_xT(t_i):
                tiles = []
                for k in range(K1):
                    t = xp.tile([128, BT], BF16, tag=f"xT{k}")
                    nc.sync.dma_start_transpose(
                        out=t[:, :],
                        in_=x_bf[t_i * BT:(t_i + 1) * BT, k * 128:(k + 1) * 128])
                    tiles.append(t)
                return tiles

            xT = load_xT(0)

            # masked weights: transpose mask chunk, multiply into weight tile
            for k in range(K1):
                mt = mp.tile([128, D2], BF16, tag="mT")
                nc.sync.dma_start_transpose(out=mt[:, :],
                                            in_=gpem_bf[:, k * 128:(k + 1) * 128])
                nc.vector.tensor_mul(wgpe[k][:, :], wgpe[k][:, :], mt[:, :])
            for k in range(K3):
                mt = mp.tile([128, D4], BF16, tag="mT")
                nc.sync.dma_start_transpose(out=mt[:, :],
                                            in_=gpim_bf[:, k * 128:(k + 1) * 128])
                nc.vector.tensor_mul(wgpi[k][:, :], wgpi[k][:, :], mt[:, :])

            # ---------------- main loop over batch tiles -------------------
            for t_i in range(NBT):
                xT_next = load_xT(t_i + 1) if t_i + 1 < NBT else None

                # L1: gpe_out[u,b] = sum_k mw_gpe[k,u] * xT[k,b]   (+bias)
                gpe_out = []
                for u in range(U2):
                    ps = psp.tile([128, BT], FP32, tag="ps")
                    for k in range(K1):
                        nc.tensor.matmul(ps[:, :],
                                         wgpe[k][:, u * 128:(u + 1) * 128],
                                         xT[k][:, :],
                                         start=(k == 0), stop=(k == K1 - 1))
                    got = ap.tile([128, BT], BF16, tag=f"gpe_out{u}")
                    nc.scalar.activation(got[:, :], ps[:, :], Act.Identity,
                                         bias=gpeb_sb[:, u:u + 1])
                    gpe_out.append(got)

                # L2: gpi_out[v,b] = sum_k mw_gpi[k,v] * gpi_in[k,b] (+bias)
                gpi_out = []
                for v in range(V4):
                    ps = psp.tile([128, BT], FP32, tag="ps")
                    for k in range(K3):
                        rhs = xT[k] if k < K1 else gpe_out[k - K1]
                        nc.tensor.matmul(ps[:, :],
                                         wgpi[k][:, v * 128:(v + 1) * 128],
                                         rhs[:, :],
                                         start=(k == 0), stop=(k == K3 - 1))
                    gio = ap.tile([128, BT], BF16, tag=f"gpi_out{v}")
                    nc.scalar.activation(gio[:, :], ps[:, :], Act.Identity,
                                         bias=gpib_sb[:, v:v + 1])
                    gpi_out.append(gio)

                # L3: h1 = relu(gpi_out @ w1 + b1)
                h1 = []
                for m in range(M5):
                    ps = psp.tile([128, BT], FP32, tag="ps")
                    for k in range(V4):
                        nc.tensor.matmul(ps[:, :],
                                         w1s[k][:, m * 128:(m + 1) * 128],
                                         gpi_out[k][:, :],
                                         start=(k == 0), stop=(k == V4 - 1))
                    hm = ap.tile([128, BT], BF16, tag=f"h1_{m}")
                    nc.scalar.activation(hm[:, :], ps[:, :], Act.Relu,
                                         bias=b1_sb[:, m:m + 1])
                    h1.append(hm)

                # L4: h2 = relu(h1 @ w2 + b2)
                h2 = []
                for m in range(M5):
                    ps = psp.tile([128, BT], FP32, tag="ps")
                    for k in range(M5):
                        nc.tensor.matmul(ps[:, :],
                                         w2s[k][:, m * 128:(m + 1) * 128],
                                         h1[k][:, :],
                                         start=(k == 0), stop=(k == M5 - 1))
                    hm = ap.tile([128, BT], BF16, tag=f"h2_{m}")
                    nc.scalar.activation(hm[:, :], ps[:, :], Act.Relu,
                                         bias=b2_sb[:, m:m + 1])
                    h2.append(hm)

                # L5: out = relu(h2 @ w3 + b3), [6, BT] f32
                ps5 = ps5p.tile([A, BT], FP32, tag="ps5")
                for k in range(M5):
                    nc.tensor.matmul(ps5[:, :], w3s[k][:, :], h2[k][:, :],
                                     start=(k == 0), stop=(k == M5 - 1))
                osb = op.tile([A, BT], FP32, tag="osb")
                nc.scalar.activation(osb[:, :], ps5[:, :], Act.Relu,
                                     bias=b3_sb[:, 0:1])
                nc.scalar.dma_start(out=o_d[:, t_i * BT:(t_i + 1) * BT],
                                    in_=osb[:, :])

                if xT_next is not None:
                    xT = xT_next

    nc.finalize()
    return nc


def _get_nc():
    if "nc" not in _CACHE:
        _CACHE["nc"] = _build()
    return _CACHE["nc"]


def _run(inputs, trace=False):
    from concourse.bass_utils import run_bass_kernel_spmd

    nc = _get_nc()
    shared = {k: np.ascontiguousarray(v, dtype=np.float32)
              for k, v in inputs.items() if k != "x"}
    x = np.ascontiguousarray(inputs["x"], dtype=np.float32)
    in_maps = [dict(shared, x=x[c * BS:(c + 1) * BS]) for c in range(NCORES)]
    res = run_bass_kernel_spmd(nc, in_maps, list(range(NCORES)), trace=trace)
    out = np.concatenate(
        [np.asarray(res.results[c]["out"]).T for c in range(NCORES)], axis=0)
    return out.astype(np.float32), res


def kernel(**inputs):
    out, _ = _run(inputs, trace=False)
    return out


# revision 2
# speedup vs baseline: 1.0731x; 1.0731x over previous
"""CTBG circuit kernel for Trainium2, data-parallel over batch on 8 NeuronCores.

Network (per reference):
  gpe_out = x @ (gpe_w * gpe_mask.T) + gpe_b              [B, 1536]
  gpi_in  = concat([x, gpe_out], -1)                      [B, 3072]
  gpi_out = gpi_in @ (gpi_w * gpi_mask.T) + gpi_b         [B, 1536]
  h1 = relu(gpi_out @ w1 + b1); h2 = relu(h1 @ w2 + b2)
  out = relu(h2 @ w3 + b3)                                [B, 6]

Per-core dataflow (feature-major activations, bf16 compute, f32 accumulate):
  - SWDGE (gpsimd) queue carries ONLY the bf16 DRAM stagings (x tiles and
    mask column-chunks, each its own DRAM tensor so there are no false
    WAR deps), ordered by when the consumer needs them.
  - weights load as f32 on the scalar HWDGE queue; DVE multiplies them with
    the xbar-transposed masks (mixed f32*bf16 -> bf16) into resident tiles.
  - SP HWDGE queue carries only DMA-xbar transposes (x and masks).
  - matmul chain keeps activations feature-major; ScalarE drains PSUM with
    bias (+relu for the MLP) straight to bf16 tiles feeding the next layer.
  - output written as [6, 2048] f32 per core; host transposes + concats.
"""

import numpy as np

NCORES = 8
B = 16384
BS = B // NCORES          # 2048 rows per core
BT = 512                  # batch tile (matmul free dim)
NBT = BS // BT            # 4
D1 = 1536                 # gpe input dim
D2 = 1536                 # gpe output dim
D3 = 3072                 # gpi input dim
D4 = 1536                 # gpi output dim
H = 512                   # mlp hidden
A = 6                     # action dim

K1 = D1 // 128            # 12
U2 = D2 // 128            # 12
K3 = D3 // 128            # 24
V4 = D4 // 128            # 12
M5 = H // 128             # 4

GPE_SPLIT = 2             # gpe mask staged in column halves
GPI_SPLIT = 4             # gpi mask staged in column quarters

_CACHE = {}


def _build():
    import concourse.bacc as bacc
    import concourse.tile as tile
    from concourse import mybir
    from concourse.masks import make_identity

    FP32 = mybir.dt.float32
    BF16 = mybir.dt.bfloat16
    Act = mybir.ActivationFunctionType

    nc = bacc.Bacc(None)

    x_d = nc.dram_tensor("x", [BS, D1], FP32, kind="ExternalInput")
    gpem_d = nc.dram_tensor("gpe_mask", [D2, D1], FP32, kind="ExternalInput")
    gpew_d = nc.dram_tensor("gpe_w", [D1, D2], FP32, kind="ExternalInput")
    gpeb_d = nc.dram_tensor("gpe_b", [D2], FP32, kind="ExternalInput")
    gpim_d = nc.dram_tensor("gpi_mask", [D4, D3], FP32, kind="ExternalInput")
    gpiw_d = nc.dram_tensor("gpi_w", [D3, D4], FP32, kind="ExternalInput")
    gpib_d = nc.dram_tensor("gpi_b", [D4], FP32, kind="ExternalInput")
    w1_d = nc.dram_tensor("w1", [D4, H], FP32, kind="ExternalInput")
    b1_d = nc.dram_tensor("b1", [H], FP32, kind="ExternalInput")
    w2_d = nc.dram_tensor("w2", [H, H], FP32, kind="ExternalInput")
    b2_d = nc.dram_tensor("b2", [H], FP32, kind="ExternalInput")
    w3_d = nc.dram_tensor("w3", [H, A], FP32, kind="ExternalInput")
    b3_d = nc.dram_tensor("b3", [A], FP32, kind="ExternalInput")
    o_d = nc.dram_tensor("out", [A, BS], FP32, kind="ExternalOutput")

    # bf16 DRAM staging, one tensor per independently-written chunk
    x_bf = [nc.dram_tensor(f"x_bf{t}", [BT, D1], BF16) for t in range(NBT)]
    gw1 = D1 // GPE_SPLIT
    gpem_bf = [nc.dram_tensor(f"gpem_bf{h}", [D2, gw1], BF16)
               for h in range(GPE_SPLIT)]
    gw2 = D3 // GPI_SPLIT
    gpim_bf = [nc.dram_tensor(f"gpim_bf{q}", [D4, gw2], BF16)
               for q in range(GPI_SPLIT)]

    with tile.TileContext(nc) as tc:
        with (
            tc.tile_pool(name="wpool", bufs=1) as wp,        # persistent weights
            tc.tile_pool(name="wfpool", bufs=2) as wfp,      # transient f32 weight rows
            tc.tile_pool(name="mpool", bufs=2) as mp,        # transient maskT tiles
            tc.tile_pool(name="xpool", bufs=2) as xp,        # xT double-buffered
            tc.tile_pool(name="apool", bufs=1) as ap,        # activations
            tc.tile_pool(name="opool", bufs=2) as op,        # output staging
            tc.tile_pool(name="pspool", bufs=4, space="PSUM") as psp,
            tc.tile_pool(name="ps5pool", bufs=2, space="PSUM") as ps5p,
        ):
            # ------------- SWDGE (gpsimd): bf16 stagings, critical first ----
            nc.gpsimd.dma_start(out=x_bf[0][:, :], in_=x_d[0:BT, :])
            for h in range(GPE_SPLIT):
                nc.gpsimd.dma_start(out=gpem_bf[h][:, :],
                                    in_=gpem_d[:, h * gw1:(h + 1) * gw1])
            for q in range(GPI_SPLIT):
                nc.gpsimd.dma_start(out=gpim_bf[q][:, :],
                                    in_=gpim_d[:, q * gw2:(q + 1) * gw2])
            for t_i in range(1, NBT):
                nc.gpsimd.dma_start(out=x_bf[t_i][:, :],
                                    in_=x_d[t_i * BT:(t_i + 1) * BT, :])

            # ------------- scalar HWDGE: f32 weight rows + biases + stores --
            # biases first (tiny), then weights in consumption order
            ident = wp.tile([128, 128], FP32, tag="ident")
            make_identity(nc, ident[:, :])

            def load_bias(b_dram, n, tag):
                nat = wp.tile([max(n, 1), 128], FP32, tag=f"{tag}_nat")
                nc.scalar.dma_start(out=nat[:, :],
                                    in_=b_dram.rearrange("(c p) -> c p", p=128))
                ps = ps5p.tile([128, max(n, 1)], FP32, tag="ps_bias")
                nc.tensor.transpose(ps[:, :], nat[:, :], ident[0:n, 0:n])
                sb = wp.tile([128, max(n, 1)], FP32, tag=tag)
                nc.vector.tensor_copy(sb[:, :], ps[:, :])
                return sb

            gpeb_sb = load_bias(gpeb_d, U2, "gpeb")
            gpib_sb = load_bias(gpib_d, V4, "gpib")
            b1_sb = load_bias(b1_d, M5, "b1sb")
            b2_sb = load_bias(b2_d, M5, "b2sb")
            b3_sb = wp.tile([A, 1], FP32, tag="b3sb")
            nc.scalar.dma_start(out=b3_sb[:, :],
                                in_=b3_d.rearrange("(a one) -> a one", one=1))

            def load_w_f32(w_dram, k, width):
                t = wfp.tile([128, width], FP32, tag="wf")
                nc.scalar.dma_start(out=t[:, 0:width],
                                    in_=w_dram[k * 128:(k + 1) * 128, :])
                return t

            # ------------- SP HWDGE: xbar transposes ------------------------
            def load_xT(t_i):
                tiles = []
                for k in range(K1):
                    t = xp.tile([128, BT], BF16, tag=f"xT{k}")
                    nc.sync.dma_start_transpose(
                        out=t[:, :],
                        in_=x_bf[t_i][:, k * 128:(k + 1) * 128])
                    tiles.append(t)
                return tiles

            xT = load_xT(0)

            # masked weights: xbar-transpose mask chunk, f32 weight row * mask
            kw1 = gw1 // 128
            wgpe = []
            for k in range(K1):
                mt = mp.tile([128, D2], BF16, tag="mT")
                src = gpem_bf[k // kw1]
                nc.sync.dma_start_transpose(
                    out=mt[:, :],
                    in_=src[:, (k % kw1) * 128:(k % kw1 + 1) * 128])
                wf = load_w_f32(gpew_d, k, D2)
                t = wp.tile([128, D2], BF16, tag=f"wgpe{k}")
                nc.vector.tensor_mul(t[:, :], wf[:, 0:D2], mt[:, :])
                wgpe.append(t)

            kw2 = gw2 // 128
            wgpi = []
            for k in range(K3):
                mt = mp.tile([128, D4], BF16, tag="mT")
                src = gpim_bf[k // kw2]
                nc.sync.dma_start_transpose(
                    out=mt[:, :],
                    in_=src[:, (k % kw2) * 128:(k % kw2 + 1) * 128])
                wf = load_w_f32(gpiw_d, k, D4)
                t = wp.tile([128, D4], BF16, tag=f"wgpi{k}")
                nc.vector.tensor_mul(t[:, :], wf[:, 0:D4], mt[:, :])
                wgpi.append(t)

            def load_w_bf(w_dram, n, width, tag):
                tiles = []
                for k in range(n):
                    wf = load_w_f32(w_dram, k, width)
                    t = wp.tile([128, width], BF16, tag=f"{tag}{k}")
                    nc.vector.tensor_copy(t[:, 0:width], wf[:, 0:width])
                    tiles.append(t)
                return tiles

            w1s = load_w_bf(w1_d, V4, H, "w1_")
            w2s = load_w_bf(w2_d, M5, H, "w2_")
            w3s = load_w_bf(w3_d, M5, A, "w3_")

            # ------------- main loop over batch tiles -----------------------
            for t_i in range(NBT):
                xT_next = load_xT(t_i + 1) if t_i + 1 < NBT else None

                # L1: gpe_out[u,b] = sum_k mw_gpe[k,u] * xT[k,b]   (+bias)
                gpe_out = []
                for u in range(U2):
                    ps = psp.tile([128, BT], FP32, tag="ps")
                    for k in range(K1):
                        nc.tensor.matmul(ps[:, :],
                                         wgpe[k][:, u * 128:(u + 1) * 128],
                                         xT[k][:, :],
                                         start=(k == 0), stop=(k == K1 - 1))
                    got = ap.tile([128, BT], BF16, tag=f"gpe_out{u}")
                    nc.scalar.activation(got[:, :], ps[:, :], Act.Identity,
                                         bias=gpeb_sb[:, u:u + 1])
                    gpe_out.append(got)

                # L2: gpi_out[v,b] = sum_k mw_gpi[k,v] * gpi_in[k,b] (+bias)
                gpi_out = []
                for v in range(V4):
                    ps = psp.tile([128, BT], FP32, tag="ps")
                    for k in range(K3):
                        rhs = xT[k] if k < K1 else gpe_out[k - K1]
                        nc.tensor.matmul(ps[:, :],
                                         wgpi[k][:, v * 128:(v + 1) * 128],
                                         rhs[:, :],
                                         start=(k == 0), stop=(k == K3 - 1))
                    gio = ap.tile([128, BT], BF16, tag=f"gpi_out{v}")
                    nc.scalar.activation(gio[:, :], ps[:, :], Act.Identity,
                                         bias=gpib_sb[:, v:v + 1])
                    gpi_out.append(gio)

                # L3: h1 = relu(gpi_out @ w1 + b1)
                h1 = []
                for m in range(M5):
                    ps = psp.tile([128, BT], FP32, tag="ps")
                    for k in range(V4):
                        nc.tensor.matmul(ps[:, :],
                                         w1s[k][:, m * 128:(m + 1) * 128],
                                         gpi_out[k][:, :],
                                         start=(k == 0), stop=(k == V4 - 1))
                    hm = ap.tile([128, BT], BF16, tag=f"h1_{m}")
                    nc.scalar.activation(hm[:, :], ps[:, :], Act.Relu,
                                         bias=b1_sb[:, m:m + 1])
                    h1.append(hm)

                # L4: h2 = relu(h1 @ w2 + b2)
                h2 = []
                for m in range(M5):
                    ps = psp.tile([128, BT], FP32, tag="ps")
                    for k in range(M5):
                        nc.tensor.matmul(ps[:, :],
                                         w2s[k][:, m * 128:(m + 1) * 128],
                                         h1[k][:, :],
                                         start=(k == 0), stop=(k == M5 - 1))
                    hm = ap.tile([128, BT], BF16, tag=f"h2_{m}")
                    nc.scalar.activation(hm[:, :], ps[:, :], Act.Relu,
                                         bias=b2_sb[:, m:m + 1])
                    h2.append(hm)

                # L5: out = relu(h2 @ w3 + b3), [6, BT] f32
                ps5 = ps5p.tile([A, BT], FP32, tag="ps5")
                for k in range(M5):
                    nc.tensor.matmul(ps5[:, :], w3s[k][:, :], h2[k][:, :],
                                     start=(k == 0), stop=(k == M5 - 1))
                osb = op.tile([A, BT], FP32, tag="osb")
                nc.scalar.activation(osb[:, :], ps5[:, :], Act.Relu,
                                     bias=b3_sb[:, 0:1])
                nc.scalar.dma_start(out=o_d[:, t_i * BT:(t_i + 1) * BT],
                                    in_=osb[:, :])

                if xT_next is not None:
                    xT = xT_next

    nc.finalize()
    return nc


def _get_nc():
    if "nc" not in _CACHE:
        _CACHE["nc"] = _build()
    return _CACHE["nc"]


def _run(inputs, trace=False):
    from concourse.bass_utils import run_bass_kernel_spmd

    nc = _get_nc()
    shared = {k: np.ascontiguousarray(v, dtype=np.float32)
              for k, v in inputs.items() if k != "x"}
    x = np.ascontiguousarray(inputs["x"], dtype=np.float32)
    in_maps = [dict(shared, x=x[c * BS:(c + 1) * BS]) for c in range(NCORES)]
    res = run_bass_kernel_spmd(nc, in_maps, list(range(NCORES)), trace=trace)
    out = np.concatenate(
        [np.asarray(res.results[c]["out"]).T for c in range(NCORES)], axis=0)
    return out.astype(np.float32), res


def kernel(**inputs):
    out, _ = _run(inputs, trace=False)
    return out


# revision 7
# speedup vs baseline: 1.3499x; 1.2580x over previous
"""CTBG circuit kernel for Trainium2, data-parallel over batch on 8 NeuronCores.

Network (per reference):
  gpe_out = x @ (gpe_w * gpe_mask.T) + gpe_b              [B, 1536]
  gpi_in  = concat([x, gpe_out], -1)                      [B, 3072]
  gpi_out = gpi_in @ (gpi_w * gpi_mask.T) + gpi_b         [B, 1536]
  h1 = relu(gpi_out @ w1 + b1); h2 = relu(h1 @ w2 + b2)
  out = relu(h2 @ w3 + b3)                                [B, 6]

Per-core dataflow (feature-major activations, bf16 compute, f32 accumulate):
  - NO DMA-xbar transposes (transpose<->copy xbar-mode transitions serialize
    the whole DMA subsystem on trn2); every transpose runs on the
    TensorEngine via identity matmuls instead, overlapped with loads.
  - x and masks stream in as bf16 row-tiles (SWDGE cast-DMA straight to
    SBUF, no DRAM staging); weights stream as f32 rows on the scalar HWDGE
    queue, cast to resident bf16 tiles by DVE.
  - masked weights: PE-transpose each 128x128 mask block into PSUM, then
    DVE multiplies it into the resident bf16 weight tile in place.
  - matmul chain keeps activations feature-major; ScalarE drains PSUM with
    bias (+relu for the MLP) straight to bf16 tiles feeding the next layer.
  - output written as [6, 2048] f32 per core; host transposes + concats.
"""

import numpy as np

NCORES = 8
B = 16384
BS = B // NCORES          # 2048 rows per core
BT = 512                  # batch tile (matmul free dim)
NBT = BS // BT            # 4
D1 = 1536                 # gpe input dim
D2 = 1536                 # gpe output dim
D3 = 3072                 # gpi input dim
D4 = 1536                 # gpi output dim
H = 512                   # mlp hidden
A = 6                     # action dim

K1 = D1 // 128            # 12
U2 = D2 // 128            # 12
K3 = D3 // 128            # 24
V4 = D4 // 128            # 12
M5 = H // 128             # 4

_CACHE = {}


def _build():
    import concourse.bacc as bacc
    import concourse.tile as tile
    from concourse import mybir
    from concourse.masks import make_identity

    FP32 = mybir.dt.float32
    BF16 = mybir.dt.bfloat16
    Act = mybir.ActivationFunctionType

    nc = bacc.Bacc(None)

    x_d = nc.dram_tensor("x", [BS, D1], FP32, kind="ExternalInput")
    gpem_d = nc.dram_tensor("gpe_mask", [D2, D1], FP32, kind="ExternalInput")
    gpew_d = nc.dram_tensor("gpe_w", [D1, D2], FP32, kind="ExternalInput")
    gpeb_d = nc.dram_tensor("gpe_b", [D2], FP32, kind="ExternalInput")
    gpim_d = nc.dram_tensor("gpi_mask", [D4, D3], FP32, kind="ExternalInput")
    gpiw_d = nc.dram_tensor("gpi_w", [D3, D4], FP32, kind="ExternalInput")
    gpib_d = nc.dram_tensor("gpi_b", [D4], FP32, kind="ExternalInput")
    w1_d = nc.dram_tensor("w1", [D4, H], FP32, kind="ExternalInput")
    b1_d = nc.dram_tensor("b1", [H], FP32, kind="ExternalInput")
    w2_d = nc.dram_tensor("w2", [H, H], FP32, kind="ExternalInput")
    b2_d = nc.dram_tensor("b2", [H], FP32, kind="ExternalInput")
    w3_d = nc.dram_tensor("w3", [H, A], FP32, kind="ExternalInput")
    b3_d = nc.dram_tensor("b3", [A], FP32, kind="ExternalInput")
    o_d = nc.dram_tensor("out", [A, BS], FP32, kind="ExternalOutput")

    with tile.TileContext(nc) as tc:
        with (
            tc.tile_pool(name="wpool", bufs=1) as wp,        # persistent weights
            tc.tile_pool(name="wfpool", bufs=2) as wfp,      # transient f32 weight half-rows
            tc.tile_pool(name="mpool", bufs=2) as mp,        # transient mask row-tiles
            tc.tile_pool(name="xrpool", bufs=2) as xrp,      # transient x row-tiles
            tc.tile_pool(name="xpool", bufs=2) as xp,        # xT double-buffered
            tc.tile_pool(name="apool", bufs=1) as ap,        # activations
            tc.tile_pool(name="opool", bufs=1) as op,        # output staging
            tc.tile_pool(name="pspool", bufs=3, space="PSUM") as psp,
            tc.tile_pool(name="pstpool", bufs=3, space="PSUM") as pstp,
            tc.tile_pool(name="ps5pool", bufs=2, space="PSUM") as ps5p,
        ):
            # ---------------- SWDGE (gpsimd): bf16 cast loads ---------------
            # order = consumption order: x tile0 rows, gpe mask, gpi mask,
            # remaining x rows
            xrow0 = []
            for r in range(BT // 128):
                t = xrp.tile([128, D1], BF16, tag="xr")
                nc.gpsimd.dma_start(out=t[:, :], in_=x_d[r * 128:(r + 1) * 128, :])
                xrow0.append(t)

            gpem_rows = []
            for u0 in range(U2):
                t = mp.tile([128, D1], BF16, tag="mrow")
                nc.gpsimd.dma_start(out=t[:, :],
                                    in_=gpem_d[u0 * 128:(u0 + 1) * 128, :])
                gpem_rows.append(t)

            gpim_rows = []          # (v0, half) -> tile, loaded in v0-major order
            for v0 in range(V4):
                for hh in range(2):
                    t = mp.tile([128, D3 // 2], BF16, tag="mrow")
                    nc.gpsimd.dma_start(
                        out=t[:, :],
                        in_=gpim_d[v0 * 128:(v0 + 1) * 128,
                                   hh * (D3 // 2):(hh + 1) * (D3 // 2)])
                    gpim_rows.append(t)

            xrow_rest = []
            for t_i in range(1, NBT):
                rows = []
                for r in range(BT // 128):
                    g = t_i * (BT // 128) + r
                    t = xrp.tile([128, D1], BF16, tag="xr")
                    nc.gpsimd.dma_start(out=t[:, :],
                                        in_=x_d[g * 128:(g + 1) * 128, :])
                    rows.append(t)
                xrow_rest.append(rows)

            # ---------------- scalar HWDGE: biases + f32 weight halves ------
            ident = wp.tile([128, 128], FP32, tag="ident")
            make_identity(nc, ident[:, :])
            identb = wp.tile([128, 128], BF16, tag="identb")
            make_identity(nc, identb[:, :])

            def load_bias(b_dram, n, tag):
                nat = wp.tile([max(n, 1), 128], FP32, tag=f"{tag}_nat")
                nc.sync.dma_start(out=nat[:, :],
                                    in_=b_dram.rearrange("(c p) -> c p", p=128))
                ps = pstp.tile([128, max(n, 1)], FP32, tag="pst")
                nc.tensor.transpose(ps[:, :], nat[:, :], ident[0:n, 0:n])
                sb = wp.tile([128, max(n, 1)], FP32, tag=tag)
                nc.vector.tensor_copy(sb[:, :], ps[:, :])
                return sb

            gpeb_sb = load_bias(gpeb_d, U2, "gpeb")
            gpib_sb = load_bias(gpib_d, V4, "gpib")
            b1_sb = load_bias(b1_d, M5, "b1sb")
            b2_sb = load_bias(b2_d, M5, "b2sb")
            b3_sb = wp.tile([A, 1], FP32, tag="b3sb")
            nc.sync.dma_start(out=b3_sb[:, :],
                                in_=b3_d.rearrange("(a one) -> a one", one=1))

            def load_w_bf(w_dram, n, width, tag, halves=2):
                """f32 rows on scalar HWDGE (in `halves` column chunks) ->
                DVE cast into a resident bf16 tile."""
                tiles = []
                hw = width // halves
                for k in range(n):
                    t = wp.tile([128, width], BF16, tag=f"{tag}{k}")
                    for hh in range(halves):
                        wf = wfp.tile([128, hw], FP32, tag="wf")
                        nc.sync.dma_start(
                            out=wf[:, 0:hw],
                            in_=w_dram[k * 128:(k + 1) * 128,
                                       hh * hw:(hh + 1) * hw])
                        nc.vector.tensor_copy(t[:, hh * hw:(hh + 1) * hw],
                                              wf[:, 0:hw])
                    tiles.append(t)
                return tiles

            wgpe = load_w_bf(gpew_d, K1, D2, "wgpe")

            # ---------------- PE transposes -------------------------------
            def prep_xT(rows):
                """x row-tiles [128b, D1] -> xT chunk tiles [128i, BT]."""
                tiles = []
                for c in range(K1):
                    t = xp.tile([128, BT], BF16, tag=f"xT{c}")
                    tiles.append(t)
                for r, xrow in enumerate(rows):
                    for c in range(K1):
                        ps = pstp.tile([128, 128], BF16, tag="pst")
                        nc.tensor.transpose(ps[:, :],
                                            xrow[:, c * 128:(c + 1) * 128],
                                            identb[:, :])
                        nc.vector.tensor_copy(
                            tiles[c][:, r * 128:(r + 1) * 128], ps[:, :])
                return tiles

            def prep_mask(rows_for, n_out, n_k, wtiles):
                """PE-transpose mask blocks, DVE-multiply into weight tiles."""
                for o0 in range(n_out):
                    row, col0 = rows_for(o0)
                    # row covers mask[o0*128:(o0+1)*128, col0:col0+ncols]
                    ncols = row.shape[-1]
                    for cc in range(ncols // 128):
                        c = col0 // 128 + cc
                        ps = pstp.tile([128, 128], BF16, tag="pst")
                        nc.tensor.transpose(ps[:, :],
                                            row[:, cc * 128:(cc + 1) * 128],
                                            identb[:, :])
                        nc.vector.tensor_mul(
                            wtiles[c][:, o0 * 128:(o0 + 1) * 128],
                            wtiles[c][:, o0 * 128:(o0 + 1) * 128],
                            ps[:, :])

            xT = prep_xT(xrow0)
            prep_mask(lambda u0: (gpem_rows[u0], 0), U2, K1, wgpe)

            # gpi + mlp weights stream while L1 runs; emitted after the gpe
            # prep so the DVE FIFO (casts) can't head-of-line-block it
            wgpi = load_w_bf(gpiw_d, K3, D4, "wgpi")
            w1s = load_w_bf(w1_d, V4, H, "w1_", halves=1)
            w2s = load_w_bf(w2_d, M5, H, "w2_", halves=1)
            w3s = load_w_bf(w3_d, M5, A, "w3_", halves=1)

            # ---------------- main loop over batch tiles -------------------
            for t_i in range(NBT):
                # L1: gpe_out[u,b] = sum_k mw_gpe[k,u] * xT[k,b]   (+bias)
                gpe_out = []
                for u in range(U2):
                    ps = psp.tile([128, BT], FP32, tag="ps")
                    for k in range(K1):
                        nc.tensor.matmul(ps[:, :],
                                         wgpe[k][:, u * 128:(u + 1) * 128],
                                         xT[k][:, :],
                                         start=(k == 0), stop=(k == K1 - 1))
                    got = ap.tile([128, BT], BF16, tag=f"gpe_out{u}")
                    nc.scalar.activation(got[:, :], ps[:, :], Act.Identity,
                                         bias=gpeb_sb[:, u:u + 1])
                    gpe_out.append(got)

                if t_i == 0:
                    # gpi masked weights: needed first by L2(t0).
                    # consume tiles in exact load order (v0-major, half inner)
                    # so the mrow pool slots recycle without stalling SWDGE.
                    for v0 in range(V4):
                        for hh in range(2):
                            row = gpim_rows[2 * v0 + hh]
                            for cc in range(K3 // 2):
                                c = hh * (K3 // 2) + cc
                                pst = pstp.tile([128, 128], BF16, tag="pst")
                                nc.tensor.transpose(
                                    pst[:, :],
                                    row[:, cc * 128:(cc + 1) * 128],
                                    identb[:, :])
                                nc.vector.tensor_mul(
                                    wgpi[c][:, v0 * 128:(v0 + 1) * 128],
                                    wgpi[c][:, v0 * 128:(v0 + 1) * 128],
                                    pst[:, :])
                    xT_next = prep_xT(xrow_rest[0])
                elif t_i + 1 < NBT:
                    xT_next = prep_xT(xrow_rest[t_i])
                else:
                    xT_next = None

                # L2: gpi_out[v,b] = sum_k mw_gpi[k,v] * gpi_in[k,b] (+bias)
                gpi_out = []
                for v in range(V4):
                    ps = psp.tile([128, BT], FP32, tag="ps")
                    for k in range(K3):
                        rhs = xT[k] if k < K1 else gpe_out[k - K1]
                        nc.tensor.matmul(ps[:, :],
                                         wgpi[k][:, v * 128:(v + 1) * 128],
                                         rhs[:, :],
                                         start=(k == 0), stop=(k == K3 - 1))
                    gio = ap.tile([128, BT], BF16, tag=f"gpi_out{v}")
                    nc.scalar.activation(gio[:, :], ps[:, :], Act.Identity,
                                         bias=gpib_sb[:, v:v + 1])
                    gpi_out.append(gio)

                # L3: h1 = relu(gpi_out @ w1 + b1)
                h1 = []
                for m in range(M5):
                    ps = psp.tile([128, BT], FP32, tag="ps")
                    for k in range(V4):
                        nc.tensor.matmul(ps[:, :],
                                         w1s[k][:, m * 128:(m + 1) * 128],
                                         gpi_out[k][:, :],
                                         start=(k == 0), stop=(k == V4 - 1))
                    hm = ap.tile([128, BT], BF16, tag=f"h1_{m}")
                    nc.scalar.activation(hm[:, :], ps[:, :], Act.Relu,
                                         bias=b1_sb[:, m:m + 1])
                    h1.append(hm)

                # L4: h2 = relu(h1 @ w2 + b2)
                h2 = []
                for m in range(M5):
                    ps = psp.tile([128, BT], FP32, tag="ps")
                    for k in range(M5):
                        nc.tensor.matmul(ps[:, :],
                                         w2s[k][:, m * 128:(m + 1) * 128],
                                         h1[k][:, :],
                                         start=(k == 0), stop=(k == M5 - 1))
                    hm = ap.tile([128, BT], BF16, tag=f"h2_{m}")
                    nc.scalar.activation(hm[:, :], ps[:, :], Act.Relu,
                                         bias=b2_sb[:, m:m + 1])
                    h2.append(hm)

                # L5: out = relu(h2 @ w3 + b3), [6, BT] f32
                ps5 = ps5p.tile([A, BT], FP32, tag="ps5")
                for k in range(M5):
                    nc.tensor.matmul(ps5[:, :], w3s[k][:, :], h2[k][:, :],
                                     start=(k == 0), stop=(k == M5 - 1))
                osb = op.tile([A, BT], FP32, tag="osb")
                nc.scalar.activation(osb[:, :], ps5[:, :], Act.Relu,
                                     bias=b3_sb[:, 0:1])
                nc.sync.dma_start(out=o_d[:, t_i * BT:(t_i + 1) * BT],
                                    in_=osb[:, :])

                if xT_next is not None:
                    xT = xT_next

    nc.finalize()
    return nc


def _get_nc():
    if "nc" not in _CACHE:
        _CACHE["nc"] = _build()
    return _CACHE["nc"]


def _run(inputs, trace=False):
    from concourse.bass_utils import run_bass_kernel_spmd

    nc = _get_nc()
    shared = {k: np.ascontiguousarray(v, dtype=np.float32)
              for k, v in inputs.items() if k != "x"}
    x = np.ascontiguousarray(inputs["x"], dtype=np.float32)
    in_maps = [dict(shared, x=x[c * BS:(c + 1) * BS]) for c in range(NCORES)]
    res = run_bass_kernel_spmd(nc, in_maps, list(range(NCORES)), trace=trace)
    out = np.concatenate(
        [np.asarray(res.results[c]["out"]).T for c in range(NCORES)], axis=0)
    return out.astype(np.float32), res


def kernel(**inputs):
    out, _ = _run(inputs, trace=False)
    return out
